# revision 4
# baseline (speedup 1.0000x reference)
"""TRN2 Bass kernel for nn_Conv2d_62826781606523 (LUT-conv, K_LUT=2).

Math per table t=(co,p,f), weights w[t,0:4], inputs a=E[b,p,f], bb=E[b,p,sel2]:
    out_t = c0 + c1*a + c2*bb + c3*a*bb         (butterfly of w)
    out[b,co,p] = sum_f out_t
Host-side weight-only preprocessing (offline-foldable):
    bias[co,p] = sum_f c0;   c12 = c1 + scatter_add(c2 over sel2)
so   out[b,co,p] = bias + sum_f (c3*bb + c12) * a.

v6 (from v5's trace): the two HWDGE rings (Sync + Scalar) are separate
DMA queues that transfer concurrently, and each ring idles ~1.3µs
between transfers — so the input streams are SPLIT across both rings
(bv/ar/consts on Sync, wc on Scalar) to double effective front
bandwidth.  Octet 0 chunk-pipelined [3,6]; octets 1-2 whole-octet ops
(min DVE op count); octet 3 tapers t into [3,3,2,1] with the bias
matmul opening each octet's psum group, so the tail is one 1-ninth
matmul + psum copy + out DMA.  bv is fp8 + Scalar expansion (halves
front-loaded bytes; Scalar is otherwise idle).  GpSimd does no
elementwise work (measured: DVE TT and GpSimd TT stall each other).
"""
import numpy as np
import ml_dtypes

import concourse.bass as bass
import concourse.bacc as bacc
import concourse.mybir as mybir
from concourse.tile import TileContext
from concourse.bass_utils import run_bass_kernel_spmd

# problem constants (hardcoded per task contract)
B, CIN, COUT, KS, H, W = 4, 16, 32, 3, 32, 32
HOUT = WOUT = 30
P = HOUT * WOUT          # 900
F = CIN * KS * KS        # 144
NCORE = 8
PPC = 113                # padded p positions per core
NB = PPC * B             # 452 columns (b-major: col = b*PPC + p)
NOCT = 4                 # co octets of 8
NINTH = 9                # f = 9 * 16
LANES = 128              # (co_local 8) x (f16)
PSB = 512                # psum col block per octet (bank-aligned)
BF16 = mybir.dt.bfloat16
FP8 = mybir.dt.float8e4
F32 = mybir.dt.float32

PCNT = [113, 113, 113, 113, 112, 112, 112, 112]
PSTART = np.concatenate([[0], np.cumsum(PCNT)[:-1]]).astype(int)

WNB = NINTH * PPC        # 1017 weight cols per half (c3 | c12)
CH = 3                   # expansion chunk (ninths)
NCHUNK = NINTH // CH
O0_CH = [3, 6]           # octet-0 ninth chunks (pipeline fast-start)
O3_CH = [3, 3, 2, 1]     # last-octet t chunks (short tail)

_cache = {}


def _build():
    nc = bacc.Bacc()
    d_wc = [nc.dram_tensor(f"wc_{o}", [LANES, 2 * WNB], BF16,
                           kind="ExternalInput") for o in range(NOCT)]
    d_bv = [nc.dram_tensor(f"bv_{o}", [LANES, NINTH * NB], FP8,
                           kind="ExternalInput") for o in range(NOCT)]
    d_ar = nc.dram_tensor("ar", [LANES, NINTH * NB], BF16, kind="ExternalInput")
    d_bias = nc.dram_tensor("biasv", [8, NOCT * PPC], BF16, kind="ExternalInput")
    d_c16 = nc.dram_tensor("cst16", [LANES, 16], BF16, kind="ExternalInput")
    d_out = nc.dram_tensor("out", [8, NOCT * NB], F32, kind="ExternalOutput")

    mul = mybir.AluOpType.mult
    add = mybir.AluOpType.add

    with TileContext(nc) as tc:
        with (
            tc.tile_pool(name="cst", bufs=1) as cst,
            tc.tile_pool(name="io", bufs=2) as io,
            tc.tile_pool(name="wk", bufs=2) as wk,
            tc.psum_pool(name="ps", bufs=1) as ps,
        ):
            # resident tiles
            ar = cst.tile([LANES, NINTH * NB], BF16, name="art")
            biast = cst.tile([8, NOCT * PPC], BF16, name="biastt")
            c16 = cst.tile([LANES, 16], BF16, name="c16t")
            psum = ps.tile([8, NOCT * PSB], F32, name="psumt")
            out_sb = cst.tile([8, NOCT * NB], F32, name="outsb")
            sred = c16[:, 0:8]
            ident8 = c16[0:8, 8:16]

            bv8 = [None] * NOCT
            wc = [None] * NOCT
            bvx = [None] * NOCT
            bv8[0] = io.tile([LANES, NINTH * NB], FP8, tag="bv8", name="bv8_0")
            wc[0] = io.tile([LANES, 2 * WNB], BF16, tag="wc", name="wc_0")
            bv8[1] = io.tile([LANES, NINTH * NB], FP8, tag="bv8", name="bv8_1")
            wc[1] = io.tile([LANES, 2 * WNB], BF16, tag="wc", name="wc_1")

            # prologue: minimum DMA slot count (each slot costs ~1.3µs of
            # ring dead time): 4 big transfers on Sync, consts on Scalar
            n0 = O0_CH[0] * NB
            nc.sync.dma_start(bv8[0][:], d_bv[0][:])
            nc.sync.dma_start(ar[:, 0:n0], d_ar[:, 0:n0])
            nc.sync.dma_start(bv8[1][:], d_bv[1][:])
            nc.sync.dma_start(ar[:, n0:NINTH * NB], d_ar[:, n0:NINTH * NB])
            nc.scalar.dma_start(wc[0][:], d_wc[0][:])
            nc.scalar.dma_start(c16[:], d_c16[:])
            nc.scalar.dma_start(biast[:], d_bias[:])

            def expand(j, nlo, nhi):
                cs = bass.ds(nlo * NB, (nhi - nlo) * NB)
                nc.scalar.copy(bvx[j][:, cs], bv8[j][:, cs])

            def make_views(j):
                q_t = wk.tile([LANES, NINTH * NB], BF16, tag="q")
                t_t = wk.tile([LANES, NINTH * NB], BF16, tag="t")
                q4 = q_t[:].rearrange("l (n b p) -> l n b p", b=B, p=PPC)
                bv4 = bvx[j][:].rearrange("l (n b p) -> l n b p", b=B, p=PPC)
                wcr = wc[j][:].rearrange("l (w n p) -> l w n p", w=2, p=PPC)
                return q_t, t_t, q4, bv4, wcr

            def q_ops(v, nlo, nhi):
                q_t, t_t, q4, bv4, wcr = v
                nsl = slice(nlo, nhi)
                w3c = wcr[:, 0, nsl].unsqueeze(2) \
                    .broadcast_to([LANES, nhi - nlo, B, PPC])
                c12c = wcr[:, 1, nsl].unsqueeze(2) \
                    .broadcast_to([LANES, nhi - nlo, B, PPC])
                nc.vector.tensor_tensor(q4[:, nsl], bv4[:, nsl], w3c, mul)
                nc.vector.tensor_tensor(q4[:, nsl], q4[:, nsl], c12c, add)

            def t_op(v, nlo, nhi):
                q_t, t_t = v[0], v[1]
                cs = bass.ds(nlo * NB, (nhi - nlo) * NB)
                nc.vector.tensor_tensor(t_t[:, cs], q_t[:, cs], ar[:, cs], mul)

            def bias_mm(o):
                biasb = biast[:].rearrange("r (o p) -> r o p", p=PPC)[:, o] \
                    .unsqueeze(1).broadcast_to([8, B, PPC])
                nc.tensor.matmul(psum[:, o * PSB:o * PSB + NB], ident8, biasb,
                                 start=True, stop=False, skip_group_check=True)

            def mm_ninths(v, o, nlo, nhi):
                t_t = v[1]
                pr = slice(o * PSB, o * PSB + NB)
                for n in range(nlo, nhi):
                    nc.tensor.matmul(
                        psum[:, pr], sred, t_t[:, bass.ts(n, NB)],
                        start=False, stop=(n == NINTH - 1),
                        skip_group_check=True)

            def finalize(o):
                nc.scalar.copy(out_sb[:, bass.ts(o, NB)],
                               psum[:, o * PSB:o * PSB + NB])
                nc.scalar.dma_start(d_out[:, bass.ts(o, NB)],
                                    out_sb[:, bass.ts(o, NB)])

            # ---- octet 0: chunk-pipelined ----
            bvx[0] = wk.tile([LANES, NINTH * NB], BF16, tag="bvx", name="bvx_0")
            expand(0, 0, O0_CH[0])
            nc.scalar.dma_start(wc[1][:], d_wc[1][:])
            expand(0, O0_CH[0], NINTH)
            v0 = make_views(0)
            bias_mm(0)
            lo = 0
            for ch in O0_CH:
                q_ops(v0, lo, lo + ch)
                t_op(v0, lo, lo + ch)
                mm_ninths(v0, 0, lo, lo + ch)
                lo += ch
            # expansion for octet 1 (runs as bv1 lands)
            bvx[1] = wk.tile([LANES, NINTH * NB], BF16, tag="bvx", name="bvx_1")
            for c in range(NCHUNK):
                expand(1, c * CH, (c + 1) * CH)
            finalize(0)

            # ---- octets 1..3 ----
            for o in range(1, NOCT):
                if o + 1 < NOCT:
                    bv8[o + 1] = io.tile([LANES, NINTH * NB], FP8, tag="bv8", name=f"bv8_{o+1}")
                    wc[o + 1] = io.tile([LANES, 2 * WNB], BF16, tag="wc", name=f"wc_{o+1}")
                    nc.sync.dma_start(bv8[o + 1][:], d_bv[o + 1][:])
                    nc.scalar.dma_start(wc[o + 1][:], d_wc[o + 1][:])
                    bvx[o + 1] = wk.tile([LANES, NINTH * NB], BF16, tag="bvx", name=f"bvx_{o+1}")
                    for c in range(NCHUNK):
                        expand(o + 1, c * CH, (c + 1) * CH)
                v = make_views(o)
                bias_mm(o)
                if o < NOCT - 1:
                    q_ops(v, 0, NINTH)
                    t_op(v, 0, NINTH)
                    mm_ninths(v, o, 0, NINTH)
                else:
                    # fully chunk-pipelined last octet: PE reduction starts
                    # ~2µs earlier, shrinking the post-DVE matmul backlog
                    for c in range(NCHUNK):
                        q_ops(v, c * CH, (c + 1) * CH)
                        t_op(v, c * CH, (c + 1) * CH)
                        mm_ninths(v, o, c * CH, (c + 1) * CH)
                finalize(o)
    nc.finalize()
    return nc


def _prep(x, weight, mask):
    x = np.ascontiguousarray(np.asarray(x, dtype=np.float32))
    weight = np.ascontiguousarray(np.asarray(weight, dtype=np.float32))
    mask = np.asarray(mask, dtype=np.int64)
    bf = ml_dtypes.bfloat16
    f8 = ml_dtypes.float8_e4m3fn

    # within-receptive-field index of LUT input 2
    m = mask.reshape(COUT, P, F, 2, 3)
    pr = (np.arange(P) // WOUT)[None, :, None]
    pc = (np.arange(P) % WOUT)[None, :, None]
    g = (m[..., 0] * KS + (m[..., 1] - pr[..., None])) * KS + (m[..., 2] - pc[..., None])
    sel2 = g[..., 1].astype(np.int64)                # (COUT,P,F)

    # im2col E[b,p,f]
    E = np.empty((B, P, F), dtype=np.float32)
    xv = x.reshape(B, CIN, H, W)
    for gg in range(F):
        cch, rem = divmod(gg, KS * KS)
        ddr, ddc = divmod(rem, KS)
        E[:, :, gg] = xv[:, cch, ddr:ddr + HOUT, ddc:ddc + WOUT].reshape(B, P)

    # gather of input-2 values: bvals[b,co,p,f] = E[b,p,sel2[co,p,f]]
    flat_idx = (np.arange(P)[None, :, None] * F + sel2).reshape(-1)
    bvals = E.reshape(B, P * F)[:, flat_idx].reshape(B, COUT, P, F)

    # weight-only preprocessing: butterfly + scatter + bias (offline-foldable)
    w4 = weight.reshape(COUT, P, F, 4)
    w0, w1, w2, w3_ = w4[..., 0], w4[..., 1], w4[..., 2], w4[..., 3]
    c0 = w0 + w1 + w2 + w3_
    c1 = -w0 - w1 + w2 + w3_
    c2 = -w0 + w1 - w2 + w3_
    c3 = w0 - w1 - w2 + w3_
    bias = c0.sum(-1)                                # (COUT,P)
    c12 = c1.copy()
    base = (np.arange(COUT * P) * F)[:, None]
    np.add.at(c12.reshape(-1), (base + sel2.reshape(COUT * P, F)).ravel(),
              c2.reshape(COUT * P, F).ravel())

    # one-hot reduction stationary packed with identity8: cst16
    cst16 = np.zeros((LANES, 16), dtype=bf)
    cst16[np.arange(LANES), np.arange(LANES) // 16] = 1.0
    cst16[0:8, 8:16] = np.eye(8, dtype=bf)

    in_maps = []
    for k in range(NCORE):
        ps_, ncnt = PSTART[k], PCNT[k]
        sl = slice(ps_, ps_ + ncnt)
        Ek = np.zeros((B, PPC, F), dtype=np.float32)
        Ek[:, :ncnt] = E[:, sl]
        bvk = np.zeros((B, COUT, PPC, F), dtype=np.float32)
        bvk[:, :, :ncnt] = bvals[:, :, sl]
        c3k = np.zeros((COUT, PPC, F), dtype=np.float32)
        c3k[:, :ncnt] = c3[:, sl]
        c12k = np.zeros((COUT, PPC, F), dtype=np.float32)
        c12k[:, :ncnt] = c12[:, sl]
        # bias laid out (r=co_local, o, p)
        biask = np.zeros((8, NOCT, PPC), dtype=np.float32)
        biask[:, :, :ncnt] = bias[:, sl].reshape(NOCT, 8, ncnt).transpose(1, 0, 2)

        im = {"cst16": cst16,
              "biasv": biask.reshape(8, NOCT * PPC).astype(bf)}
        E_l = Ek.reshape(B, PPC, NINTH, 16)          # (b,p,n,j)
        im["ar"] = np.ascontiguousarray(
            np.broadcast_to(E_l.transpose(3, 2, 0, 1)[None], (8, 16, NINTH, B, PPC))
        ).reshape(LANES, NINTH * NB).astype(bf)
        for o in range(NOCT):
            cosl = slice(8 * o, 8 * o + 8)
            bvo = bvk[:, cosl].reshape(B, 8, PPC, NINTH, 16)
            im[f"bv_{o}"] = np.ascontiguousarray(
                bvo.transpose(1, 4, 3, 0, 2)).reshape(LANES, NINTH * NB).astype(f8)
            wcx = np.empty((8, 16, 2, NINTH, PPC), dtype=np.float32)
            wcx[:, :, 0] = c3k[cosl].reshape(8, PPC, NINTH, 16).transpose(0, 3, 2, 1)
            wcx[:, :, 1] = c12k[cosl].reshape(8, PPC, NINTH, 16).transpose(0, 3, 2, 1)
            im[f"wc_{o}"] = wcx.reshape(LANES, 2 * WNB).astype(bf)
        in_maps.append(im)
    return in_maps


def kernel(x, weight, mask):
    if "nc" not in _cache:
        _cache["nc"] = _build()
    nc = _cache["nc"]
    in_maps = _prep(x, weight, mask)
    res = run_bass_kernel_spmd(nc, in_maps, core_ids=list(range(NCORE)))
    out = np.empty((B, COUT, HOUT, WOUT), dtype=np.float32)
    ov = out.reshape(B, COUT, P)
    for k in range(NCORE):
        dev = res.results[k]["out"].reshape(8, NOCT, B, PPC)     # (r, o, b, p)
        full = dev.transpose(2, 1, 0, 3).reshape(B, COUT, PPC)   # co = 8o + r
        ov[:, :, PSTART[k]:PSTART[k] + PCNT[k]] = full[:, :, :PCNT[k]]
    return out


if __name__ == "__main__":
    print("kernel module ok")
